# revision 53
# baseline (speedup 1.0000x reference)
"""Multi-head attention (B=8, S=2048, E=1024, H=16, D=64) on 8 TRN2 NeuronCores.

Sharding: data parallel over batch — core b computes batch b end to end.

Per-core device kernel (all matmuls fp16, fp32 accumulation):
  scores^T[j, i] = K^T_tile.T @ Q^T        (contraction over d=64, PE; one
      single-bank PSUM tile [128, 512] per (j-tile, head))
  expS = exp(scores / 8)                   split across two engines:
      9/16 of j-tiles: ScalarE activation (PSUM -> SBUF fp16)
      7/16 of j-tiles: DVE copies scores PSUM -> SBUF fp32, then GPSIMD
          tensor_tensor pow computes (e^{1/8})^s -> SBUF fp16
  acc[i, 65]   += expS[j, i_tile].T @ [V_h | 1]   (PE; expS tile is the
      *stationary* operand, the narrow [V|1] (65 cols) is the moving one,
      so each 128x128x65 step streams only 65 columns; col 64 = softmax sums.
      The 4 i-tile accumulators of a head pack into one PSUM bank and share
      one start/stop accumulation group - start zeroes the whole 2KB bank.)
  norm[i, d]    = acc[:, 0:64] * (1 / acc[:, 64])  (DVE broadcast multiply)
  concatT[e, i] = dma_transpose(norm)      (xbar transpose back to e-major)
  out[i, o]     = concatT_chunk.T @ W_out^T_chunk + b_out   (PE; both
      256-wide o-chunks of a projection unit pack into one PSUM bank)

The emission is software-pipelined over (i-chunk, head-pair, j-tile) tasks:
exp/bounce lags QK by one task, gpsimd pow by two, attn@V by twelve. The score
and projection tiles share a 6-bank PSUM rotation (~2.7 tasks deep) so the
in-order PE stream rides out scheduling jitter on the exp engines. Output
projection for i-chunk ii and the accumulator transposes are dripped across
the tasks of chunk ii+1; input DMAs are ordered by first-use deadline.

Host does layout prep only: head-transposes Q/K to [H, D, S], appends the ones
column to V, transposes W_out, broadcasts the bias row, casts to fp16, and
scatters/gathers per batch.
"""

import sys

if "/opt/trn_rl_repo" not in sys.path:
    sys.path.insert(0, "/opt/trn_rl_repo")

from contextlib import ExitStack

import numpy as np

B, S, E, H, D = 8, 2048, 1024, 16, 64
P = 128            # partitions
IC = 512           # query-position chunk per inner loop
NI = S // IC       # 4 i-chunks
NJ = S // P        # 16 key-position tiles (= pipeline tasks per pair)
NK = E // P        # 8 contraction chunks in the output projection
NO = 2             # e_out chunks of 512
NP_ = H // 2       # 8 head pairs
NIT = IC // P      # 4 i-tiles per chunk
SCALE = 1.0 / 8.0  # 1/sqrt(D)
BASE = float(np.exp(np.float64(0.125)))  # pow base: BASE**s == exp(s/8)
POOL_JT = (1, 3, 5, 7, 9, 12, 14)  # j-tiles whose exp runs on GPSIMD via pow
LAG_EXP, LAG_POW, LAG_AV = 1, 2, 12

_NC_CACHE = {}


def _build_nc():
    import concourse.mybir as mybir
    import concourse.tile as tile
    from concourse import bacc

    f32 = mybir.dt.float32
    f16 = mybir.dt.float16
    Exp = mybir.ActivationFunctionType.Exp
    mult = mybir.AluOpType.mult
    add = mybir.AluOpType.add
    pow_op = mybir.AluOpType.pow

    nc = bacc.Bacc(
        "TRN2",
        target_bir_lowering=False,
        debug=False,
        enable_asserts=False,
        num_devices=8,
    )

    qt_d = nc.dram_tensor("qt", [H, D, S], f16, kind="ExternalInput")
    kt_d = nc.dram_tensor("kt", [H, D, S], f16, kind="ExternalInput")
    # [jt, p, h, 65]: per s-tile row tile, per partition, per head: [v(64) | 1]
    vt_d = nc.dram_tensor("vt", [NJ, P, H, D + 1], f16, kind="ExternalInput")
    wt_d = nc.dram_tensor("wt", [E, E], f16, kind="ExternalInput")
    bi_d = nc.dram_tensor("bias", [P, E], f32, kind="ExternalInput")
    out_d = nc.dram_tensor("out", [S, E], f16, kind="ExternalOutput")

    with tile.TileContext(nc) as tc, ExitStack() as ctx:
        const = ctx.enter_context(tc.tile_pool(name="const", bufs=1))
        qpool = ctx.enter_context(tc.tile_pool(name="qpool", bufs=2))
        epool = ctx.enter_context(tc.tile_pool(name="epool", bufs=21))
        e2pool = ctx.enter_context(tc.tile_pool(name="e2pool", bufs=9))
        bpool = ctx.enter_context(tc.tile_pool(name="bpool", bufs=4))
        stpool = ctx.enter_context(tc.tile_pool(name="stpool", bufs=2))
        ccpool = ctx.enter_context(tc.tile_pool(name="ccpool", bufs=2))
        rpool = ctx.enter_context(tc.tile_pool(name="rpool", bufs=4))
        opool = ctx.enter_context(tc.tile_pool(name="opool", bufs=3))
        spool = ctx.enter_context(tc.tile_pool(name="spool", bufs=6, space="PSUM"))
        acpool = ctx.enter_context(tc.tile_pool(name="acpool", bufs=2, space="PSUM"))

        # --- persistent tiles, DMAs chunked in first-use order ---------------
        kt_all = const.tile([P, NP_, S], f16)
        vt_all = const.tile([P, NJ, H, D + 1], f16)
        wt_all = const.tile([P, NK, E], f16)
        bias_bc = const.tile([P, E], f32)
        pbase = const.tile([P, 2, IC], f32)

        kt_r = kt_d.ap().rearrange("(hp hh) d s -> (hh d) hp s", hh=2)
        qt_r = qt_d.ap().rearrange("(hp hh) d s -> (hh d) hp s", hh=2)
        vt_r = vt_d.ap().rearrange("jt p h e -> p jt h e")

        qt_tiles = {}
        cc_tiles = {}

        def get_cc(ii):
            if ii not in cc_tiles:
                cc_tiles[ii] = ccpool.tile([P, NK, IC], f16, tag="cc",
                                           name=f"cc_{ii}")
            return cc_tiles[ii]

        def load_qt(ii):
            t = qpool.tile([P, NP_, IC], f16)
            isl = slice(ii * IC, (ii + 1) * IC)
            for p in range(NP_):
                nc.sync.dma_start(t[:, p, :], qt_r[:, p, isl])
            qt_tiles[ii] = t

        # first pair's operands first so compute starts within a few µs.
        # kt pair 0 and the first vt tiles go via the ACT HWDGE ring so they
        # run in parallel with the qt loads on the SP ring.
        qt0 = qpool.tile([P, NP_, IC], f16, name="qt0")
        nc.sync.dma_start(qt0[:, 0, :], qt_r[:, 0, 0:IC])
        nc.scalar.dma_start(kt_all[:, 0, 0:128], kt_r[:, 0, 0:128])
        nc.scalar.dma_start(kt_all[:, 0, 128:512], kt_r[:, 0, 128:512])
        nc.scalar.dma_start(kt_all[:, 0, 512:S], kt_r[:, 0, 512:S])
        for p in range(1, NP_):
            nc.sync.dma_start(qt0[:, p, :], qt_r[:, p, 0:IC])
        qt_tiles[0] = qt0
        nc.vector.memset(pbase[:], BASE)
        # preload the exp activation table during the DMA prologue so the
        # first real exp doesn't pay the ~1.3us ACT_TABLE_LOAD mid-pipeline
        tdummy = rpool.tile([1, 1], f16, tag="td", name="tdummy")
        nc.scalar.activation(tdummy[:], pbase[0:1, 0, 0:1], Exp, scale=SCALE)
        # interleave vt j-tiles (needed by attn@V from ~task 12 onward) with
        # kt pairs (pair p needed at task 16p); wt/bias are projection-only
        # and load last.
        for jt in range(0, 4):
            nc.sync.dma_start(vt_all[:, jt, :, :], vt_r[:, jt, :, :])
        nc.sync.dma_start(kt_all[:, 1, :], kt_r[:, 1, :])
        for jt in range(4, 10):
            nc.sync.dma_start(vt_all[:, jt, :, :], vt_r[:, jt, :, :])
        nc.sync.dma_start(kt_all[:, 2, :], kt_r[:, 2, :])
        for jt in range(10, NJ):
            nc.sync.dma_start(vt_all[:, jt, :, :], vt_r[:, jt, :, :])
        for p in range(3, NP_):
            nc.sync.dma_start(kt_all[:, p, :], kt_r[:, p, :])
        nc.sync.dma_start(wt_all[:], wt_d.ap().rearrange("(ko ki) o -> ki ko o", ki=P))
        nc.sync.dma_start(bias_bc[:], bi_d.ap())

        # --- pipelined emission ----------------------------------------------
        sc_tiles = {}      # (ii, p, jt, hh) -> psum score tile [P, IC]
        bounce_tiles = {}  # (ii, p, jt, hh) -> sbuf fp32 score tile
        ex_tiles = {}      # (ii, p, jt, hh) -> sbuf fp16 exp tile
        accs = {}          # (ii, p) -> (accT0, accT1): [P, NIT, 65] psum
        pending_proj = []  # deferred projection units from finished i-chunks
        pending_tail = []  # deferred transpose closures

        def emit_qk(ii, p, jt):
            qt_ii = qt_tiles[ii]
            jsl = slice(jt * P, (jt + 1) * P)
            for hb, hh in ((0, 0), (64, 1)):
                sc = spool.tile([P, IC], f32, tag="sc",
                                name=f"sc{hh}_{ii}_{p}_{jt}")
                nc.tensor.matmul(
                    sc[:],
                    kt_all[hb : hb + 64, p, jsl],
                    qt_ii[hb : hb + 64, p, :],
                    start=True, stop=True,
                )
                sc_tiles[(ii, p, jt, hh)] = sc

        def emit_exp(ii, p, jt):
            """Stage 2: ACT exp for scalar j-tiles, DVE bounce for pool ones."""
            if jt in POOL_JT:
                # both heads bounce into one [P, 2, IC] staging tile so the
                # gpsimd pow below is a single instruction per j-tile
                b = bpool.tile([P, 2, IC], f32, tag="bc",
                               name=f"bc_{ii}_{p}_{jt}")
                for hh in range(2):
                    sc = sc_tiles.pop((ii, p, jt, hh))
                    nc.vector.tensor_copy(b[:, hh, :], sc[:])
                bounce_tiles[(ii, p, jt)] = b
            else:
                for hh in range(2):
                    sc = sc_tiles.pop((ii, p, jt, hh))
                    e = epool.tile([P, IC], f16, tag="ex",
                                   name=f"ex{hh}_{ii}_{p}_{jt}")
                    nc.scalar.activation(e[:], sc[:], Exp, scale=SCALE)
                    ex_tiles[(ii, p, jt, hh)] = e

        def emit_pow(ii, p, jt):
            """Stage 3: GPSIMD pow for pool j-tiles."""
            if jt not in POOL_JT:
                return
            b = bounce_tiles.pop((ii, p, jt))
            e = e2pool.tile([P, 2, IC], f16, tag="ex2",
                           name=f"px_{ii}_{p}_{jt}")
            nc.gpsimd.tensor_tensor(e[:], pbase[:], b[:], pow_op)
            ex_tiles[(ii, p, jt, 0)] = e[:, 0, :]
            ex_tiles[(ii, p, jt, 1)] = e[:, 1, :]

        def emit_av(ii, p, jt):
            """Stage 4: attn@V with expS stationary, [V|1] moving."""
            if (ii, p) not in accs:
                accs[(ii, p)] = tuple(
                    acpool.tile([P, NIT, D + 1], f32, tag="acc",
                                name=f"acc{hh}_{ii}_{p}")
                    for hh in range(2)
                )
            for hh in range(2):
                e = ex_tiles.pop((ii, p, jt, hh))
                eap = e[:] if hasattr(e, "tile_pool") or hasattr(e, "pool") else e
                h = 2 * p + hh
                for it in range(NIT):
                    # One accumulation group per PSUM bank: start zeroes the
                    # whole 2KB zero region (all 4 it-slices), so only the
                    # bank's first matmul starts and its last stops.
                    nc.tensor.matmul(
                        accs[(ii, p)][hh][:, it, :],
                        eap[:, it * P : (it + 1) * P],
                        vt_all[:, jt, h, :],
                        start=(jt == 0 and it == 0),
                        stop=(jt == NJ - 1 and it == NIT - 1),
                    )

        def emit_tail(ii, p):
            """Normalize pair accumulators; transposes drip into later tasks."""
            accT = accs.pop((ii, p))
            st = stpool.tile([P, NIT, P], f16, tag="st", name=f"st_{ii}_{p}")
            for hh in range(2):
                rc = rpool.tile([P, NIT, 1], f32, tag="rc",
                                name=f"rc{hh}_{ii}_{p}")
                nc.vector.reciprocal(rc[:], accT[hh][:, :, D : D + 1])
                # broadcast multiply: rc's last dim repeats via stride 0,
                # normalizing all 4 i-tiles in a single DVE op
                nc.vector.tensor_tensor(
                    st[:, :, hh * D : (hh + 1) * D],
                    accT[hh][:, :, 0:D],
                    rc[:].broadcast_to([P, NIT, D]),
                    mult,
                )

            cc = get_cc(ii)

            def transpose(it, p=p, st=st, cc=cc):
                nc.sync.dma_start_transpose(
                    cc[:, p, it * P : (it + 1) * P], st[:, it, :]
                )

            if ii == NI - 1:
                for it in range(NIT):
                    transpose(it)
            else:
                pending_tail.extend(
                    (lambda it=it: transpose(it)) for it in range(NIT)
                )

        def emit_proj(ii, it, o):
            cc = cc_tiles[ii] if it < NIT - 1 or o < NO - 1 else cc_tiles.pop(ii)
            i0 = ii * IC + it * P
            osl = slice(o * 512, (o + 1) * 512)
            # pp packs two 256-wide o-chunks into one PSUM bank; the bank's
            # first matmul starts (zeroing the whole bank) and its last stops.
            pp = spool.tile([P, 2, 256], f32, tag="sc", name=f"pp_{ii}_{it}_{o}")
            for o2 in range(2):
                o0 = o * 512 + o2 * 256
                for k in range(NK):
                    nc.tensor.matmul(
                        pp[:, o2, :],
                        cc[:, k, it * P : (it + 1) * P],
                        wt_all[:, k, o0 : o0 + 256],
                        start=(k == 0 and o2 == 0),
                        stop=(k == NK - 1 and o2 == 1),
                    )
            ob = opool.tile([P, 512], f16)
            nc.vector.tensor_tensor(
                ob[:].rearrange("p (a b) -> p a b", a=2),
                pp[:],
                bias_bc[:, osl].rearrange("p (a b) -> p a b", a=2),
                add,
            )
            nc.sync.dma_start(out_d.ap()[i0 : i0 + P, osl], ob[:])

        tasks = [
            (ii, p, jt)
            for ii in range(NI)
            for p in range(NP_)
            for jt in range(NJ)
        ]
        NT = len(tasks)

        def run_stages(k):
            """Emit the lagged pipeline stages for loop position k."""
            if 0 <= k - LAG_EXP < NT:
                emit_exp(*tasks[k - LAG_EXP])
            if 0 <= k - LAG_POW < NT:
                emit_pow(*tasks[k - LAG_POW])
            if 0 <= k - LAG_AV < NT:
                ii, p, jt = tasks[k - LAG_AV]
                emit_av(ii, p, jt)
                if jt == NJ - 1:
                    emit_tail(ii, p)
                    if p == NP_ - 1:
                        pending_proj.extend(
                            (ii, it, o)
                            for it in range(NIT)
                            for o in range(NO)
                        )
            if pending_tail:
                pending_tail.pop(0)()
            if pending_proj and k % 8 == 2:
                emit_proj(*pending_proj.pop(0))

        for k, (ii, p, jt) in enumerate(tasks):
            if p == 2 and jt == 0 and ii + 1 < NI:
                load_qt(ii + 1)
            emit_qk(ii, p, jt)
            run_stages(k)
        for k in range(NT, NT + LAG_AV):
            run_stages(k)
        while pending_tail:
            pending_tail.pop(0)()
        while pending_proj:
            emit_proj(*pending_proj.pop(0))

    nc.compile()
    return nc


def get_nc():
    if "nc" not in _NC_CACHE:
        _NC_CACHE["nc"] = _build_nc()
    return _NC_CACHE["nc"]


def make_in_maps(values, keys, queries, W_out, b_out):
    f16 = np.float16
    q = np.ascontiguousarray(
        np.asarray(queries, dtype=np.float32)
        .astype(f16)
        .reshape(B, S, H, D)
        .transpose(0, 2, 3, 1)
    )  # [B, H, D, S]
    k = np.ascontiguousarray(
        np.asarray(keys, dtype=np.float32)
        .astype(f16)
        .reshape(B, S, H, D)
        .transpose(0, 2, 3, 1)
    )
    v = np.asarray(values, dtype=np.float32).reshape(B, S, H, D)
    vt = np.empty((B, S, H, D + 1), dtype=f16)
    vt[..., :D] = v.astype(f16)
    vt[..., D] = np.float32(1.0)
    vt = vt.reshape(B, NJ, P, H, D + 1)
    wt = np.ascontiguousarray(np.asarray(W_out, dtype=np.float32).T).astype(f16)
    bias = np.ascontiguousarray(
        np.broadcast_to(
            np.asarray(b_out, dtype=np.float32).reshape(1, E), (P, E)
        )
    )
    return [
        {"qt": q[b], "kt": k[b], "vt": vt[b], "wt": wt, "bias": bias}
        for b in range(B)
    ]


def kernel(values, keys, queries, W_out, b_out):
    from concourse.bass_utils import run_bass_kernel_spmd

    nc = get_nc()
    in_maps = make_in_maps(values, keys, queries, W_out, b_out)
    res = run_bass_kernel_spmd(nc, in_maps, core_ids=list(range(8)))
    out = np.stack([res.results[b]["out"] for b in range(B)], axis=0)
    return np.ascontiguousarray(out.astype(np.float32))
